# revision 49
# baseline (speedup 1.0000x reference)
"""EnhancedVLAD Trainium2 kernel — pure data-parallel over 8 NeuronCores.

Math (validated against the reference):
  xn = x / max(||x||_c, eps)
  assign = softmax_k(conv_w @ xn + conv_b)          (logits bounded, no max-sub)
  agg[k,c] = sum_n assign[k,n] * xn[c,n] ;  mass[k] = sum_n assign[k,n]
  vlad = agg - centroids * mass[:,None]
  Ghost down-weighting and attention row-scales are strictly positive per-row
  scalars, so they cancel in the per-row L2 normalization; ghost rows are
  dropped. Each kept row is unit-norm, so the global norm is exactly
  sqrt(64) = 8  =>  out = rownorm(vlad[:64]) / 8.

Design (~158us vs the SWDGE-cast + xbar-transpose baseline at ~289us):
  * x is cast to bf16 AND pre-transposed on the host; the device does two
    plain HWDGE loads per half-batch unit (x_nat for stage-1 weights, xT for
    stage-2 rhs + column norms).  No SWDGE cast, no xbar transpose, no
    event-sem absorber machinery.  HBM traffic: 2 x 16.8 MB/core.  The x_nat
    DRAM layout is h-major so every load is 16KB-contiguous per partition.
  * One manual InstLoadActFuncSet pins ACT to natural_log_exp_and_others
    (Exp+Ln+Square); the norm/epilogue rsqrt is Exp(-0.5*Ln(n2)) so no other
    table set is ever touched (the first-match chooser otherwise thrashes
    ~23 table reloads = ~35us).
  * Column norms: per-tile squares split ACT (Square+accum_out) / POOL
    (tensor_mul + DVE reduce) / DVE (scalar_tensor_tensor fused
    square+accum_out); per-tile ops beat batched chunks (scheduling).
  * Softmax per 4-tile group with a SHORT tail (prescale->Exp->reduce->
    recip->mul->sg) so group g's stage-2 matmuls overlap group g+1's Exp;
    unit-wide batching lengthens the critical chain and loses ~wall time.

Per-core pipeline (B_local=4 batches as 8 half-batch units of 2048 cols):
  x_nat [128c, 4q, 2048n] bf16 | xT [128n, 16t, 512c] bf16   (HWDGE loads)
  stage1: lg[128n, 4, 72] = sum_q x_tile^T @ cwt_q            (PE, PSUM)
  softmax: lgs = lg * inv(bcast); ex = Exp(lgs) bf16; se = reduce;
           sc = (1/se)*inv; sg[128, 4, 64] = ex * sc(bcast)
  stage2: agg[64, 512] += sg_t^T @ xT_t ; mass += sg_t^T @ ||x||_t  (PE)
  epilogue: vlad = agg - cent*mass; out = vlad * exp(-0.5 ln(rownorm^2)) / 8
"""

import os
import sys

for _p in ("/opt/trn_rl_repo", "/opt/pypackages"):
    if _p not in sys.path and os.path.isdir(_p):
        sys.path.insert(0, _p)

import numpy as np
import ml_dtypes

import concourse.bass as bass
import concourse.bacc as bacc
import concourse.mybir as mybir
from concourse import tile
from concourse.bass_utils import run_bass_kernel_spmd
from concourse.alu_op_type import AluOpType as OP

F32 = mybir.dt.float32
BF16 = mybir.dt.bfloat16
AF = mybir.ActivationFunctionType

N_CORES = 8
B_TOTAL, C, N = 32, 512, 4096
B_LOC = B_TOTAL // N_CORES          # 4
T_CL, K_CL = 72, 64                 # clusters (with ghosts), kept clusters
NQ = C // 128                       # 4 c-chunks
N_H = N // 2                        # half-batch columns
NT_H = N_H // 128                   # 16 n-tiles per unit
NT = N // 128                       # 32 n-tiles per batch
GRP = 4                             # n-tiles per PSUM logits group
EPS = 1e-12

# square-tile engine schedule per unit (16 tiles): a=ACT Square+accum_out,
# p=POOL mul + DVE reduce, d=DVE fused square+accum (scalar_tensor_tensor).
# Fine-grained per-tile ops measured faster than batched chunks (v3 regression).
SQ_SCHED = "apdpdapdpdapdpda"  # 4xACT, 6xPOOL, 6xDVE interleaved
assert len(SQ_SCHED) == NT_H
ACT_SET_ID = 6    # natural_log_exp_and_others: covers Exp, Ln, Square


def _build_program(with_bias: bool) -> bass.Bass:
    nc = bacc.Bacc("TRN2", target_bir_lowering=False, debug=False)

    # h-major so each half-batch load reads 16KB-contiguous per partition
    # (4KB-fragmented descriptors measured ~2.5x slower on the same bytes)
    xn_d = nc.declare_dram_parameter("xnat", [B_LOC, 2, 128, NQ, N_H], BF16, isOutput=False)
    xt_d = nc.declare_dram_parameter("xt", [B_LOC, 128, NT, C], BF16, isOutput=False)
    cwt_d = nc.declare_dram_parameter("convwt", [128, NQ, T_CL], BF16, isOutput=False)
    cent_d = nc.declare_dram_parameter("cent", [K_CL, C], F32, isOutput=False)
    if with_bias:
        cb_d = nc.declare_dram_parameter("convb", [1, T_CL], BF16, isOutput=False)
    out_d = nc.declare_dram_parameter("out", [B_LOC, K_CL * C], F32, isOutput=True)

    with tile.TileContext(nc) as tc:
        with (
            tc.tile_pool(name="const", bufs=1) as constp,
            tc.tile_pool(name="xnat", bufs=4) as xnatp,
            tc.tile_pool(name="xt", bufs=4) as xtp,
            tc.tile_pool(name="strip", bufs=4) as stripp,
            tc.tile_pool(name="scr", bufs=3) as scrp,
            tc.tile_pool(name="lgs", bufs=4) as lgsp,
            tc.tile_pool(name="ex", bufs=3) as exp_pool,
            tc.tile_pool(name="sg", bufs=3) as sgp,
            tc.tile_pool(name="epi", bufs=2) as epip,
            tc.tile_pool(name="lg", bufs=5, space="PSUM") as lgp,
            tc.tile_pool(name="agg", bufs=2, space="PSUM") as aggp,
            tc.tile_pool(name="mass", bufs=1, space="PSUM") as massp,
        ):
            # Pin the ACT function table to the one set covering Exp+Ln+Square;
            # the table-load pass then inserts no per-function reloads
            # (measured 23 reloads = ~35us otherwise).
            nc.scalar.add_instruction(mybir.InstLoadActFuncSet(
                name=nc.get_next_instruction_name(), ins=[], outs=[],
                act_func_set_id=ACT_SET_ID))

            cwt = constp.tile([128, NQ, T_CL], BF16)
            nc.sync.dma_start(cwt[:], cwt_d[:])
            cent = constp.tile([K_CL, C], F32)
            nc.sync.dma_start(cent[:], cent_d[:])
            eps_b = constp.tile([128, 1], F32)
            nc.vector.memset(eps_b[:], EPS)
            if with_bias:
                ones_row = constp.tile([1, 128], BF16)
                nc.vector.memset(ones_row[:], 1.0)
                cb = constp.tile([1, T_CL], BF16)
                nc.sync.dma_start(cb[:], cb_d[:])

            agg = mass = None
            xt0_dma = None

            for u in range(2 * B_LOC):
                b, h = divmod(u, 2)
                # ---- loads: natural layout (SP ring) + transposed (ACT ring).
                # Unit 0's xt gates the whole pipeline fill: give it the full
                # DMA pool by queuing u0's x_nat behind it on the same ring
                # and holding the sync ring's first x_nat until it completes.
                x_nat = xnatp.tile([128, NQ, N_H], BF16, tag="xnat")
                xt = xtp.tile([128, NT_H, C], BF16, tag="xt")
                if u == 0:
                    xt0_dma = nc.scalar.dma_start(
                        xt[:], xt_d[b, :, 0:NT_H, :]
                    )
                    nc.scalar.dma_start(x_nat[:], xn_d[b, 0])
                else:
                    xn_dma = nc.sync.dma_start(x_nat[:], xn_d[b, h])
                    if u == 1:
                        bass._add_dep_helper(
                            xn_dma.ins, xt0_dma.ins, sync=True,
                            reason="hold sync ring until u0 xt lands",
                        )
                    nc.scalar.dma_start(
                        xt[:], xt_d[b, :, h * NT_H : (h + 1) * NT_H, :]
                    )

                # ---- per-column channel norms (ACT / POOL / DVE split) ----
                n2 = stripp.tile([128, NT_H], F32, tag="n2")
                for t in range(NT_H):
                    kind = SQ_SCHED[t]
                    if kind == "a":
                        scr = scrp.tile([128, C], BF16, tag="scrA")
                        nc.scalar.activation(
                            scr[:], xt[:, t, :], AF.Square,
                            accum_out=n2[:, t : t + 1],
                        )
                    elif kind == "p":
                        scr = scrp.tile([128, C], BF16, tag="scrP")
                        nc.gpsimd.tensor_mul(scr[:], xt[:, t, :], xt[:, t, :])
                        nc.vector.tensor_reduce(
                            n2[:, t : t + 1], scr[:], mybir.AxisListType.X, OP.add
                        )
                    else:
                        scr = scrp.tile([128, C], BF16, tag="scrD")
                        nc.vector.scalar_tensor_tensor(
                            scr[:], xt[:, t, :], 1.0, xt[:, t, :],
                            OP.bypass, OP.mult,
                            accum_out=n2[:, t : t + 1],
                        )

                # inv = n2^-0.5, nrmb = n2^+0.5 via Ln/Exp (keeps ACT on the
                # natural_log_exp_and_others table set -- no table reloads)
                nl = stripp.tile([128, NT_H], F32, tag="nl")
                nc.scalar.activation(nl[:], n2[:], AF.Ln, bias=eps_b[:])
                inv = stripp.tile([128, NT_H], F32, tag="inv")
                nc.scalar.activation(inv[:], nl[:], AF.Exp, scale=-0.5)
                nrmb = stripp.tile([128, NT_H], BF16, tag="nrmb")
                nc.scalar.activation(nrmb[:], nl[:], AF.Exp, scale=0.5)

                if h == 0:
                    agg = aggp.tile([K_CL, C], F32, tag="agg")
                    mass = massp.tile([K_CL, 1], F32, tag="mass")

                for g in range(NT_H // GRP):
                    # ---- stage 1 ----
                    lg = lgp.tile([128, GRP, T_CL], F32, tag="lg")
                    for i in range(GRP):
                        t = g * GRP + i
                        for q in range(NQ):
                            nc.tensor.matmul(
                                lg[:, i, :],
                                x_nat[:, q, bass.ts(t, 128)],
                                cwt[:, q, :],
                                start=(q == 0),
                                stop=(q == NQ - 1) if not with_bias else False,
                            )
                        if with_bias:
                            nc.tensor.matmul(
                                lg[:, i, :], ones_row[:], cb[:],
                                start=False, stop=True,
                            )
                    # ---- softmax: pre-scale by 1/||x|| then Exp ----
                    lgs = lgsp.tile([128, GRP, T_CL], F32, tag="lgs")
                    inv_b = inv[:, g * GRP : (g + 1) * GRP].unsqueeze(-1) \
                        .broadcast_to([128, GRP, T_CL])
                    nc.vector.tensor_tensor(lgs[:], lg[:], inv_b, OP.mult)
                    ex = exp_pool.tile([128, GRP, T_CL], BF16, tag="ex")
                    nc.scalar.activation(ex[:], lgs[:], AF.Exp)

                    # ---- per-group softmax tail (keeps the chain short so
                    # stage 2 of group g overlaps Exp of group g+1) ----
                    se = stripp.tile([128, GRP], F32, tag="se")
                    nc.vector.tensor_reduce(
                        se[:], ex[:], mybir.AxisListType.X, OP.add
                    )
                    sc = stripp.tile([128, GRP], F32, tag="sc")
                    nc.vector.reciprocal(sc[:], se[:])
                    nc.vector.tensor_mul(
                        sc[:], sc[:], inv[:, g * GRP : (g + 1) * GRP]
                    )
                    sg = sgp.tile([128, GRP, K_CL], BF16, tag="sg")
                    sc_b = sc[:].unsqueeze(-1).broadcast_to([128, GRP, K_CL])
                    nc.vector.tensor_tensor(sg[:], ex[:, :, 0:K_CL], sc_b, OP.mult)

                    # ---- stage 2 ----
                    for i in range(GRP):
                        t = g * GRP + i
                        tt = h * NT_H + t
                        nc.tensor.matmul(
                            agg[:], sg[:, i, :], xt[:, t, :],
                            start=(tt == 0), stop=(tt == NT - 1),
                        )
                        nc.tensor.matmul(
                            mass[:], sg[:, i, :], nrmb[:, t : t + 1],
                            start=(tt == 0), stop=(tt == NT - 1),
                        )

                if h == 1:
                    # ---- epilogue ----
                    mass_sb = epip.tile([K_CL, 1], F32, tag="mass_sb")
                    nc.vector.tensor_copy(mass_sb[:], mass[:])
                    cm = epip.tile([K_CL, C], F32, tag="cm")
                    nc.vector.tensor_scalar(cm[:], cent[:], mass_sb[:], None, OP.mult)
                    vlad = epip.tile([K_CL, C], F32, tag="vlad")
                    nc.vector.tensor_sub(vlad[:], agg[:], cm[:])

                    vsq = epip.tile([K_CL, C], BF16, tag="vsq")
                    rn2 = epip.tile([K_CL, 1], F32, tag="rn2")
                    nc.scalar.activation(vsq[:], vlad[:], AF.Square, accum_out=rn2[:])
                    rnl = epip.tile([K_CL, 1], F32, tag="rnl")
                    nc.scalar.activation(rnl[:], rn2[:], AF.Ln, bias=eps_b[0:K_CL, :])
                    rinv = epip.tile([K_CL, 1], F32, tag="rinv")
                    nc.scalar.activation(rinv[:], rnl[:], AF.Exp, scale=-0.5)

                    ob = epip.tile([K_CL, C], F32, tag="ob")
                    nc.vector.tensor_scalar(
                        ob[:], vlad[:], rinv[:], 0.125, OP.mult, OP.mult
                    )
                    nc.sync.dma_start(
                        out_d[b].rearrange("(k c) -> k c", c=C), ob[:]
                    )

    nc.compile()
    return nc


_CACHE: dict = {}


def _get_program(with_bias: bool) -> bass.Bass:
    key = ("prog", with_bias)
    if key not in _CACHE:
        _CACHE[key] = _build_program(with_bias)
    return _CACHE[key]


def _prep_params(conv_w: np.ndarray, centroids: np.ndarray):
    # conv_wT chunked: convwt[p, q, k] = conv_w[k, 128q + p]
    cwt = np.ascontiguousarray(
        conv_w.T.reshape(NQ, 128, T_CL).transpose(1, 0, 2)
    ).astype(ml_dtypes.bfloat16)
    cent = np.ascontiguousarray(centroids[:K_CL]).astype(np.float32)
    return cwt, cent


def _prep_x(x: np.ndarray):
    """Host-side bf16 cast + both device layouts.

    xnat[i][b, p, q, n] = x[4i+b, 128q+p, n]
    xt[i][b, p, t, c]   = x[4i+b, c, 128t+p]
    """
    xb = x.astype(ml_dtypes.bfloat16)  # [32, 512, 4096]
    # xnat[b, h, p, q, n] = x[b, 128q+p, h*N_H + n]
    xnat = np.ascontiguousarray(
        xb.reshape(B_TOTAL, NQ, 128, 2, N_H).transpose(0, 3, 2, 1, 4)
    )
    xt = np.ascontiguousarray(
        xb.transpose(0, 2, 1).reshape(B_TOTAL, NT, 128, C).transpose(0, 2, 1, 3)
    )
    return xnat, xt


def build_in_maps(x, centroids, conv_w, conv_b):
    with_bias = bool(np.any(np.asarray(conv_b)))
    cwt, cent = _prep_params(np.asarray(conv_w, np.float32),
                             np.asarray(centroids, np.float32))
    xnat, xt = _prep_x(np.asarray(x, np.float32))
    in_maps = []
    for i in range(N_CORES):
        m = {
            "xnat": np.ascontiguousarray(xnat[i * B_LOC : (i + 1) * B_LOC]),
            "xt": np.ascontiguousarray(xt[i * B_LOC : (i + 1) * B_LOC]),
            "convwt": cwt,
            "cent": cent,
        }
        if with_bias:
            m["convb"] = np.asarray(conv_b, np.float32).reshape(1, T_CL).astype(
                ml_dtypes.bfloat16
            )
        in_maps.append(m)
    return in_maps, with_bias


def kernel(x, centroids, conv_w, conv_b, ghost_weights, w1, b1, w2, b2) -> np.ndarray:
    in_maps, with_bias = build_in_maps(x, centroids, conv_w, conv_b)
    nc = _get_program(with_bias)
    res = run_bass_kernel_spmd(nc, in_maps, core_ids=list(range(N_CORES)))
    out = np.concatenate([r["out"] for r in res.results], axis=0)
    return np.ascontiguousarray(out.astype(np.float32))


# revision 50
# speedup vs baseline: 1.0786x; 1.0786x over previous
"""EnhancedVLAD Trainium2 kernel — pure data-parallel over 8 NeuronCores.

Math (validated against the reference):
  xn = x / max(||x||_c, eps)
  assign = softmax_k(conv_w @ xn + conv_b)          (logits bounded, no max-sub)
  agg[k,c] = sum_n assign[k,n] * xn[c,n] ;  mass[k] = sum_n assign[k,n]
  vlad = agg - centroids * mass[:,None]
  Ghost down-weighting and attention row-scales are strictly positive per-row
  scalars, so they cancel in the per-row L2 normalization; ghost rows are
  dropped. Each kept row is unit-norm, so the global norm is exactly
  sqrt(64) = 8  =>  out = rownorm(vlad[:64]) / 8.

Design (~158us vs the SWDGE-cast + xbar-transpose baseline at ~289us):
  * x is cast to bf16 AND pre-transposed on the host; the device does two
    plain HWDGE loads per half-batch unit (x_nat for stage-1 weights, xT for
    stage-2 rhs + column norms).  No SWDGE cast, no xbar transpose, no
    event-sem absorber machinery.  HBM traffic: 2 x 16.8 MB/core.  The x_nat
    DRAM layout is h-major so every load is 16KB-contiguous per partition.
  * One manual InstLoadActFuncSet pins ACT to natural_log_exp_and_others
    (Exp+Ln+Square); the norm/epilogue rsqrt is Exp(-0.5*Ln(n2)) so no other
    table set is ever touched (the first-match chooser otherwise thrashes
    ~23 table reloads = ~35us).
  * Column norms: per-tile squares split ACT (Square+accum_out) / POOL
    (tensor_mul + DVE reduce) / DVE (scalar_tensor_tensor fused
    square+accum_out); per-tile ops beat batched chunks (scheduling).
  * Softmax per 4-tile group with a SHORT tail (prescale->Exp->reduce->
    recip->mul->sg) so group g's stage-2 matmuls overlap group g+1's Exp;
    unit-wide batching lengthens the critical chain and loses ~wall time.

Per-core pipeline (B_local=4 batches as 8 half-batch units of 2048 cols):
  x_nat [128c, 4q, 2048n] bf16 | xT [128n, 16t, 512c] bf16   (HWDGE loads)
  stage1: lg[128n, 4, 72] = sum_q x_tile^T @ cwt_q            (PE, PSUM)
  softmax: lgs = lg * inv(bcast); ex = Exp(lgs) bf16; se = reduce;
           sc = (1/se)*inv; sg[128, 4, 64] = ex * sc(bcast)
  stage2: agg[64, 512] += sg_t^T @ xT_t ; mass += sg_t^T @ ||x||_t  (PE)
  epilogue: vlad = agg - cent*mass; out = vlad * exp(-0.5 ln(rownorm^2)) / 8
"""

import os
import sys

for _p in ("/opt/trn_rl_repo", "/opt/pypackages"):
    if _p not in sys.path and os.path.isdir(_p):
        sys.path.insert(0, _p)

import numpy as np
import ml_dtypes

import concourse.bass as bass
import concourse.bacc as bacc
import concourse.mybir as mybir
from concourse import tile
from concourse.bass_utils import run_bass_kernel_spmd
from concourse.alu_op_type import AluOpType as OP

F32 = mybir.dt.float32
BF16 = mybir.dt.bfloat16
AF = mybir.ActivationFunctionType

N_CORES = 8
B_TOTAL, C, N = 32, 512, 4096
B_LOC = B_TOTAL // N_CORES          # 4
T_CL, K_CL = 72, 64                 # clusters (with ghosts), kept clusters
NQ = C // 128                       # 4 c-chunks
N_H = N // 2                        # half-batch columns
NT_H = N_H // 128                   # 16 n-tiles per unit
NT = N // 128                       # 32 n-tiles per batch
GRP = 4                             # n-tiles per PSUM logits group
EPS = 1e-12

# square-tile engine schedule per unit (16 tiles): a=ACT Square+accum_out,
# p=POOL mul + DVE reduce, d=DVE fused square+accum (scalar_tensor_tensor).
# Fine-grained per-tile ops measured faster than batched chunks (v3 regression).
SQ_SCHED = "apdpdapdpdapdpda"  # 4xACT, 6xPOOL, 6xDVE interleaved
assert len(SQ_SCHED) == NT_H
ACT_SET_ID = 6    # natural_log_exp_and_others: covers Exp, Ln, Square


def _build_program(with_bias: bool) -> bass.Bass:
    nc = bacc.Bacc("TRN2", target_bir_lowering=False, debug=False)

    # h-major so each half-batch load reads 16KB-contiguous per partition
    # (4KB-fragmented descriptors measured ~2.5x slower on the same bytes)
    xn_d = nc.declare_dram_parameter("xnat", [B_LOC, 2, 128, NQ, N_H], BF16, isOutput=False)
    xt_d = nc.declare_dram_parameter("xt", [B_LOC, 128, NT, C], BF16, isOutput=False)
    cwt_d = nc.declare_dram_parameter("convwt", [128, NQ, T_CL], BF16, isOutput=False)
    cent_d = nc.declare_dram_parameter("cent", [K_CL, C], F32, isOutput=False)
    if with_bias:
        cb_d = nc.declare_dram_parameter("convb", [1, T_CL], BF16, isOutput=False)
    out_d = nc.declare_dram_parameter("out", [B_LOC, K_CL * C], F32, isOutput=True)

    with tile.TileContext(nc) as tc:
        with (
            tc.tile_pool(name="const", bufs=1) as constp,
            tc.tile_pool(name="xnat", bufs=4) as xnatp,
            tc.tile_pool(name="xt", bufs=4) as xtp,
            tc.tile_pool(name="strip", bufs=4) as stripp,
            tc.tile_pool(name="scr", bufs=3) as scrp,
            tc.tile_pool(name="lgs", bufs=4) as lgsp,
            tc.tile_pool(name="ex", bufs=3) as exp_pool,
            tc.tile_pool(name="sg", bufs=3) as sgp,
            tc.tile_pool(name="epi", bufs=2) as epip,
            tc.tile_pool(name="lg", bufs=5, space="PSUM") as lgp,
            tc.tile_pool(name="agg", bufs=2, space="PSUM") as aggp,
            tc.tile_pool(name="mass", bufs=1, space="PSUM") as massp,
        ):
            # Pin the ACT function table to the one set covering Exp+Ln+Square;
            # the table-load pass then inserts no per-function reloads
            # (measured 23 reloads = ~35us otherwise).
            nc.scalar.add_instruction(mybir.InstLoadActFuncSet(
                name=nc.get_next_instruction_name(), ins=[], outs=[],
                act_func_set_id=ACT_SET_ID))

            cwt = constp.tile([128, NQ, T_CL], BF16)
            nc.sync.dma_start(cwt[:], cwt_d[:])
            cent = constp.tile([K_CL, C], F32)
            nc.sync.dma_start(cent[:], cent_d[:])
            eps_b = constp.tile([128, 1], F32)
            nc.vector.memset(eps_b[:], EPS)
            if with_bias:
                ones_row = constp.tile([1, 128], BF16)
                nc.vector.memset(ones_row[:], 1.0)
                cb = constp.tile([1, T_CL], BF16)
                nc.sync.dma_start(cb[:], cb_d[:])

            agg = mass = None

            for u in range(2 * B_LOC):
                b, h = divmod(u, 2)
                # ---- loads: natural layout (SP ring) + transposed (ACT ring)
                x_nat = xnatp.tile([128, NQ, N_H], BF16, tag="xnat")
                nc.sync.dma_start(x_nat[:], xn_d[b, h])
                xt = xtp.tile([128, NT_H, C], BF16, tag="xt")
                nc.scalar.dma_start(
                    xt[:], xt_d[b, :, h * NT_H : (h + 1) * NT_H, :]
                )

                # ---- per-column channel norms (ACT / POOL / DVE split) ----
                n2 = stripp.tile([128, NT_H], F32, tag="n2")
                for t in range(NT_H):
                    kind = SQ_SCHED[t]
                    if kind == "a":
                        scr = scrp.tile([128, C], BF16, tag="scrA")
                        nc.scalar.activation(
                            scr[:], xt[:, t, :], AF.Square,
                            accum_out=n2[:, t : t + 1],
                        )
                    elif kind == "p":
                        scr = scrp.tile([128, C], BF16, tag="scrP")
                        nc.gpsimd.tensor_mul(scr[:], xt[:, t, :], xt[:, t, :])
                        nc.vector.tensor_reduce(
                            n2[:, t : t + 1], scr[:], mybir.AxisListType.X, OP.add
                        )
                    else:
                        scr = scrp.tile([128, C], BF16, tag="scrD")
                        nc.vector.scalar_tensor_tensor(
                            scr[:], xt[:, t, :], 1.0, xt[:, t, :],
                            OP.bypass, OP.mult,
                            accum_out=n2[:, t : t + 1],
                        )

                # inv = n2^-0.5, nrmb = n2^+0.5 via Ln/Exp (keeps ACT on the
                # natural_log_exp_and_others table set -- no table reloads)
                nl = stripp.tile([128, NT_H], F32, tag="nl")
                nc.scalar.activation(nl[:], n2[:], AF.Ln, bias=eps_b[:])
                inv = stripp.tile([128, NT_H], F32, tag="inv")
                nc.scalar.activation(inv[:], nl[:], AF.Exp, scale=-0.5)
                nrmb = stripp.tile([128, NT_H], BF16, tag="nrmb")
                nc.scalar.activation(nrmb[:], nl[:], AF.Exp, scale=0.5)

                if h == 0:
                    agg = aggp.tile([K_CL, C], F32, tag="agg")
                    mass = massp.tile([K_CL, 1], F32, tag="mass")

                for g in range(NT_H // GRP):
                    # ---- stage 1 ----
                    lg = lgp.tile([128, GRP, T_CL], F32, tag="lg")
                    for i in range(GRP):
                        t = g * GRP + i
                        for q in range(NQ):
                            nc.tensor.matmul(
                                lg[:, i, :],
                                x_nat[:, q, bass.ts(t, 128)],
                                cwt[:, q, :],
                                start=(q == 0),
                                stop=(q == NQ - 1) if not with_bias else False,
                            )
                        if with_bias:
                            nc.tensor.matmul(
                                lg[:, i, :], ones_row[:], cb[:],
                                start=False, stop=True,
                            )
                    # ---- softmax: pre-scale by 1/||x|| then Exp ----
                    lgs = lgsp.tile([128, GRP, T_CL], F32, tag="lgs")
                    inv_b = inv[:, g * GRP : (g + 1) * GRP].unsqueeze(-1) \
                        .broadcast_to([128, GRP, T_CL])
                    nc.vector.tensor_tensor(lgs[:], lg[:], inv_b, OP.mult)
                    ex = exp_pool.tile([128, GRP, T_CL], BF16, tag="ex")
                    nc.scalar.activation(ex[:], lgs[:], AF.Exp)

                    # ---- per-group softmax tail (keeps the chain short so
                    # stage 2 of group g overlaps Exp of group g+1) ----
                    se = stripp.tile([128, GRP], F32, tag="se")
                    nc.vector.tensor_reduce(
                        se[:], ex[:], mybir.AxisListType.X, OP.add
                    )
                    sc = stripp.tile([128, GRP], F32, tag="sc")
                    nc.vector.reciprocal(sc[:], se[:])
                    nc.vector.tensor_mul(
                        sc[:], sc[:], inv[:, g * GRP : (g + 1) * GRP]
                    )
                    sg = sgp.tile([128, GRP, K_CL], BF16, tag="sg")
                    sc_b = sc[:].unsqueeze(-1).broadcast_to([128, GRP, K_CL])
                    nc.vector.tensor_tensor(sg[:], ex[:, :, 0:K_CL], sc_b, OP.mult)

                    # ---- stage 2 ----
                    for i in range(GRP):
                        t = g * GRP + i
                        tt = h * NT_H + t
                        nc.tensor.matmul(
                            agg[:], sg[:, i, :], xt[:, t, :],
                            start=(tt == 0), stop=(tt == NT - 1),
                        )
                        nc.tensor.matmul(
                            mass[:], sg[:, i, :], nrmb[:, t : t + 1],
                            start=(tt == 0), stop=(tt == NT - 1),
                        )

                if h == 1:
                    # ---- epilogue ----
                    mass_sb = epip.tile([K_CL, 1], F32, tag="mass_sb")
                    nc.vector.tensor_copy(mass_sb[:], mass[:])
                    cm = epip.tile([K_CL, C], F32, tag="cm")
                    nc.vector.tensor_scalar(cm[:], cent[:], mass_sb[:], None, OP.mult)
                    vlad = epip.tile([K_CL, C], F32, tag="vlad")
                    nc.vector.tensor_sub(vlad[:], agg[:], cm[:])

                    vsq = epip.tile([K_CL, C], BF16, tag="vsq")
                    rn2 = epip.tile([K_CL, 1], F32, tag="rn2")
                    nc.scalar.activation(vsq[:], vlad[:], AF.Square, accum_out=rn2[:])
                    rnl = epip.tile([K_CL, 1], F32, tag="rnl")
                    nc.scalar.activation(rnl[:], rn2[:], AF.Ln, bias=eps_b[0:K_CL, :])
                    rinv = epip.tile([K_CL, 1], F32, tag="rinv")
                    nc.scalar.activation(rinv[:], rnl[:], AF.Exp, scale=-0.5)

                    ob = epip.tile([K_CL, C], F32, tag="ob")
                    nc.vector.tensor_scalar(
                        ob[:], vlad[:], rinv[:], 0.125, OP.mult, OP.mult
                    )
                    nc.sync.dma_start(
                        out_d[b].rearrange("(k c) -> k c", c=C), ob[:]
                    )

    nc.compile()
    return nc


_CACHE: dict = {}


def _get_program(with_bias: bool) -> bass.Bass:
    key = ("prog", with_bias)
    if key not in _CACHE:
        _CACHE[key] = _build_program(with_bias)
    return _CACHE[key]


def _prep_params(conv_w: np.ndarray, centroids: np.ndarray):
    # conv_wT chunked: convwt[p, q, k] = conv_w[k, 128q + p]
    cwt = np.ascontiguousarray(
        conv_w.T.reshape(NQ, 128, T_CL).transpose(1, 0, 2)
    ).astype(ml_dtypes.bfloat16)
    cent = np.ascontiguousarray(centroids[:K_CL]).astype(np.float32)
    return cwt, cent


def _prep_x(x: np.ndarray):
    """Host-side bf16 cast + both device layouts.

    xnat[i][b, p, q, n] = x[4i+b, 128q+p, n]
    xt[i][b, p, t, c]   = x[4i+b, c, 128t+p]
    """
    xb = x.astype(ml_dtypes.bfloat16)  # [32, 512, 4096]
    # xnat[b, h, p, q, n] = x[b, 128q+p, h*N_H + n]
    xnat = np.ascontiguousarray(
        xb.reshape(B_TOTAL, NQ, 128, 2, N_H).transpose(0, 3, 2, 1, 4)
    )
    xt = np.ascontiguousarray(
        xb.transpose(0, 2, 1).reshape(B_TOTAL, NT, 128, C).transpose(0, 2, 1, 3)
    )
    return xnat, xt


def build_in_maps(x, centroids, conv_w, conv_b):
    with_bias = bool(np.any(np.asarray(conv_b)))
    cwt, cent = _prep_params(np.asarray(conv_w, np.float32),
                             np.asarray(centroids, np.float32))
    xnat, xt = _prep_x(np.asarray(x, np.float32))
    in_maps = []
    for i in range(N_CORES):
        m = {
            "xnat": np.ascontiguousarray(xnat[i * B_LOC : (i + 1) * B_LOC]),
            "xt": np.ascontiguousarray(xt[i * B_LOC : (i + 1) * B_LOC]),
            "convwt": cwt,
            "cent": cent,
        }
        if with_bias:
            m["convb"] = np.asarray(conv_b, np.float32).reshape(1, T_CL).astype(
                ml_dtypes.bfloat16
            )
        in_maps.append(m)
    return in_maps, with_bias


def kernel(x, centroids, conv_w, conv_b, ghost_weights, w1, b1, w2, b2) -> np.ndarray:
    in_maps, with_bias = build_in_maps(x, centroids, conv_w, conv_b)
    nc = _get_program(with_bias)
    res = run_bass_kernel_spmd(nc, in_maps, core_ids=list(range(N_CORES)))
    out = np.concatenate([r["out"] for r in res.results], axis=0)
    return np.ascontiguousarray(out.astype(np.float32))
